# revision 9
# baseline (speedup 1.0000x reference)
"""NVFP4-fake-quant MLP (x@w1.T -> gelu -> @w2.T) on 8 trn2 NeuronCores.

Sharding (megatron tensor-parallel on the hidden dim):
  core c holds w1 rows [c*2048:(c+1)*2048], w2 cols [c*2048:(c+1)*2048],
  and x rows [c*1024:(c+1)*1024] (for distributed x-quantization).

Exact quantization:
  per-16-block e4m3 scales via exponent-mask + magic-number RNE;
  fp4 e2m1 rounding via 3-region clamp + magic-round decomposition.
  e2m1_value * e4m3_blockscale has <= 6 mantissa bits -> stored EXACTLY in
  bf16, so the bf16 matmuls reproduce the f32 reference; per-tensor scales
  are folded into the PSUM->SBUF copies (gelu input scale / output scale).

Dataflow:
  quantized x is transposed on-chip and AllGathered (bf16);
  h is quantized/written natural and transpose-loaded in phase 2;
  fp32 partials are ReduceScattered in 8 chunks overlapped with compute.
  The host only slices inputs and re-interleaves the RS chunk rows.
"""
import os
import sys
import numpy as np

if "/opt/trn_rl_repo" not in sys.path:
    sys.path.insert(0, "/opt/trn_rl_repo")

f32 = np.float32

B, D_IN, HID, D_OUT = 8192, 4096, 16384, 4096
NCORES = 8
BSH = B // NCORES          # 1024 x-rows quantized per core
HSH = HID // NCORES        # 2048 hidden units per core
SB = 256                   # phase-2 transpose-load superblock rows
NSB = B // SB
NBT = B // 128             # 64 b-tiles
RSCH = 8                   # reduce-scatter chunks
RSROWS = B // RSCH         # 1024 rows per RS chunk
NK1 = D_IN // 128          # 32 k-tiles, first matmul
NK2 = HSH // 128           # 16 k-tiles, second matmul

# magic round-to-nearest-even constants (f32-exact)
C_HALF = float(f32(1.5 * 2 ** 22))       # grid 0.5
C_1 = float(f32(1.5 * 2 ** 23))          # grid 1
C_1B = float(f32(1.5 * 2 ** 23 + 2.0))   # C_1 + 2
C_2 = float(f32(1.5 * 2 ** 24))          # grid 2
C_2B = float(f32(1.5 * 2 ** 24 + 4.0))   # C_2 + 4
E4M3_MAGIC = float(f32(1.5 * 2 ** 20))   # * 2^e -> magic const for step 2^(e-3)
EXPMASK = 0x7F800000
SIGNMASK = 0x80000000
ONEBITS = 0x3F800000

_BUILT = {}
USE_ERF = os.environ.get("KQ_USE_ERF", "1") == "1"


def _emit_quant(nc, mybir, pf, pb, pn, biases, src, out, c1, effmul, W):
    """Quantize src [128, W] f32 (SBUF) -> out [128, W] bf16 = sign*e2m1*bscale.

    c1: 1/(6*tensor_scale)  (float imm or [128,1] AP)
    effmul: tensor_scale    (float imm or [128,1] AP)
    biases: dict of [128,1] f32 bias tiles for the ACT magic rounds.
    """
    OP = mybir.AluOpType
    AF = mybir.ActivationFunctionType
    U32 = mybir.dt.uint32
    FP32 = mybir.dt.float32
    BF16 = mybir.dt.bfloat16
    NB = W // 16

    absv = pf.tile([128, W], FP32, tag="q_absv")
    nc.scalar.activation(absv[:], src, AF.Abs)
    amax = pn.tile([128, NB], FP32, tag="q_amax")
    nc.vector.tensor_reduce(amax[:], absv[:].rearrange("p (nb b) -> p nb b", b=16),
                            axis=mybir.AxisListType.X, op=OP.max)
    vq = pn.tile([128, NB], FP32, tag="q_vq")
    nc.vector.tensor_scalar(vq[:], amax[:], c1, None, OP.mult)
    scq = pn.tile([128, NB], FP32, tag="q_scq")
    nc.vector.tensor_scalar(scq[:].bitcast(U32), vq[:].bitcast(U32),
                            EXPMASK, None, OP.bitwise_and)
    cb = pn.tile([128, NB], FP32, tag="q_cb")
    nc.vector.tensor_scalar(cb[:], scq[:], E4M3_MAGIC, None, OP.mult)
    t4 = pn.tile([128, NB], FP32, tag="q_t4")
    nc.vector.tensor_tensor(t4[:], vq[:], cb[:], OP.add)
    bs = pn.tile([128, NB], FP32, tag="q_bs")
    nc.vector.tensor_tensor(bs[:], t4[:], cb[:], OP.subtract)
    bs16 = pn.tile([128, NB], BF16, tag="q_bs16")
    nc.vector.tensor_scalar(bs16[:], bs[:], 2.0 ** -6, None, OP.max)
    eff = pn.tile([128, NB], FP32, tag="q_eff")
    nc.vector.tensor_scalar(eff[:], bs[:], 2.0 ** -6, effmul, OP.max, OP.mult)
    rec = pn.tile([128, NB], FP32, tag="q_rec")
    nc.vector.reciprocal(rec[:], eff[:])
    r = pf.tile([128, W], FP32, tag="q_r")
    nc.vector.tensor_tensor(
        r[:].rearrange("p (nb b) -> p nb b", b=16),
        absv[:].rearrange("p (nb b) -> p nb b", b=16),
        rec[:, :, None].to_broadcast([128, NB, 16]), OP.mult)
    m1 = pf.tile([128, W], FP32, tag="q_absv")     # reuse absv slots
    nc.vector.tensor_scalar(m1[:], r[:], 2.0, None, OP.min)
    m2 = pf.tile([128, W], FP32, tag="q_m2")
    nc.vector.tensor_scalar(m2[:], r[:], 2.0, 4.0, OP.max, OP.min)
    m3 = pf.tile([128, W], FP32, tag="q_m3")
    nc.vector.tensor_scalar(m3[:], r[:], 4.0, 6.0, OP.max, OP.min)
    # RNE onto each region's grid; s1/s2 ACT (magic-add in place, then out-sub
    # with bf16 downcast), s3 via one fused DVE tensor_scalar.
    nc.scalar.activation(m1[:], m1[:], AF.Identity, bias=biases["ch"][:])
    s1 = pb.tile([128, W], BF16, tag="q_s1")
    nc.scalar.activation(s1[:], m1[:], AF.Identity, bias=biases["nch"][:])
    nc.scalar.activation(m2[:], m2[:], AF.Identity, bias=biases["c1"][:])
    s2 = pb.tile([128, W], BF16, tag="q_s2")
    nc.scalar.activation(s2[:], m2[:], AF.Identity, bias=biases["nc1b"][:])
    s3 = pb.tile([128, W], BF16, tag="q_s3")
    nc.vector.tensor_scalar(s3[:], m3[:], C_2, C_2B, OP.add, OP.subtract)
    q12 = pb.tile([128, W], BF16, tag="q_q12")
    nc.vector.tensor_tensor(q12[:], s1[:], s2[:], OP.add)
    qq = pb.tile([128, W], BF16, tag="q_qq")
    nc.vector.tensor_tensor(qq[:], q12[:], s3[:], OP.add)
    qs = pb.tile([128, W], BF16, tag="q_qs")
    nc.vector.tensor_tensor(
        qs[:].rearrange("p (nb b) -> p nb b", b=16),
        qq[:].rearrange("p (nb b) -> p nb b", b=16),
        bs16[:, :, None].to_broadcast([128, NB, 16]), OP.mult)
    sgn = pf.tile([128, W], FP32, tag="q_r")       # reuse r slots
    nc.vector.tensor_scalar(sgn[:].bitcast(U32), src.bitcast(U32),
                            SIGNMASK, ONEBITS, OP.bitwise_and, OP.bitwise_or)
    nc.vector.tensor_tensor(out, qs[:], sgn[:], OP.mult)


def _build(isc, hsc):
    from contextlib import ExitStack
    import concourse.bass as bass
    import concourse.tile as tile
    from concourse import bacc, mybir

    OP = mybir.AluOpType
    AF = mybir.ActivationFunctionType
    FP32 = mybir.dt.float32
    BF16 = mybir.dt.bfloat16

    c1x = float(f32(1.0) / (f32(6.0) * f32(isc)))
    c1h = float(f32(1.0) / (f32(6.0) * f32(hsc)))
    c1h2 = float(f32(f32(1.0) / (f32(6.0) * f32(hsc))) * f32(0.5))
    inv2688 = float(f32(1.0) / f32(2688.0))
    RG = [list(range(NCORES))]

    nc = bacc.Bacc("TRN2", target_bir_lowering=False, debug=False,
                   num_devices=NCORES)
    x_sh = nc.dram_tensor("x_sh", [BSH, D_IN], FP32, kind="ExternalInput").ap()
    w1_sh = nc.dram_tensor("w1_sh", [HSH, D_IN], FP32, kind="ExternalInput").ap()
    w2_sh = nc.dram_tensor("w2_sh", [D_OUT, HSH], FP32, kind="ExternalInput").ap()
    out_sh = nc.dram_tensor("out_sh", [BSH, D_OUT], FP32, kind="ExternalOutput").ap()

    with tile.TileContext(nc) as tc, ExitStack() as top:
        dram = top.enter_context(tc.tile_pool(name="dram", bufs=1, space="DRAM"))
        amax_stage = dram.tile([128, 2], FP32, tag="amax_stage")
        scalars_loc = dram.tile([1, 2], FP32, tag="scalars_loc")
        scales_sh = dram.tile([1, 2], FP32, tag="scales_sh", addr_space="Shared")
        xq_loc = dram.tile([BSH, D_IN], BF16, tag="xq_loc")
        xqT_loc = dram.tile([D_IN, BSH], BF16, tag="xqT_loc")
        xqT_full = dram.tile([NCORES * D_IN, BSH], BF16, tag="xqT_full",
                             addr_space="Shared")
        w1q = dram.tile([HSH, D_IN], BF16, tag="w1q")
        w2q = dram.tile([D_OUT, HSH], BF16, tag="w2q")
        hq = dram.tile([B, HSH], BF16, tag="hq")
        parts = [dram.tile([RSROWS, D_OUT], FP32, name=f"part{c}", tag=f"part{c}")
                 for c in range(RSCH)]
        rsouts = [dram.tile([128, D_OUT], FP32, name=f"rsout{c}", tag=f"rsout{c}")
                  for c in range(RSCH)]

        singles = top.enter_context(tc.tile_pool(name="singles", bufs=1))
        biases = {}
        for nm, val in [("ch", C_HALF), ("nch", -C_HALF),
                        ("c1", C_1), ("nc1b", -C_1B)]:
            bt = singles.tile([128, 1], FP32, tag=f"bias_{nm}")
            nc.vector.memset(bt[:], val)
            biases[nm] = bt

        # ================= Phase 0 =================
        with tc.tile_pool(name="p0src", bufs=2) as p0src, \
             tc.tile_pool(name="p0f", bufs=2) as p0f, \
             tc.tile_pool(name="p0b", bufs=2) as p0b, \
             tc.tile_pool(name="p0n", bufs=2) as p0n:
            # ---- x quantize (gates AG -> phase 1) ----
            for i in range(BSH // 128):
                xt = p0src.tile([128, D_IN], FP32, tag="xt")
                nc.sync.dma_start(xt[:], x_sh[i * 128:(i + 1) * 128, :])
                xo = p0src.tile([128, D_IN], BF16, tag="xo")
                for c in range(4):
                    sl = slice(c * 1024, (c + 1) * 1024)
                    _emit_quant(nc, mybir, p0f, p0b, p0n, biases,
                                xt[:, sl], xo[:, sl], c1x, float(isc), 1024)
                nc.sync.dma_start(xq_loc[i * 128:(i + 1) * 128, :], xo[:])
            # transpose xq_loc -> xqT_loc, then AllGather
            for k in range(NK1):
                xtt = p0src.tile([128, BSH], BF16, tag="xtt")
                nc.sync.dma_start(xtt[:], xq_loc[:, k * 128:(k + 1) * 128],
                                  transpose=True)
                nc.sync.dma_start(xqT_loc[k * 128:(k + 1) * 128, :], xtt[:])
            nc.gpsimd.collective_compute(
                "AllGather", OP.bypass, replica_groups=RG,
                ins=[xqT_loc[:].opt()], outs=[xqT_full[:].opt()])

            # ---- weight amaxes + tiny AllReduce(max) ----
            for wi, (wap, ntile, wcols) in enumerate(
                    [(w1_sh, HSH // 128, D_IN), (w2_sh, D_OUT // 128, HSH)]):
                acc = singles.tile([128, 1], FP32, tag=f"acc{wi}")
                for i in range(ntile):
                    wt = p0src.tile([128, D_IN], FP32, tag="xt")
                    nc.sync.dma_start(wt[:, :wcols],
                                      wap[i * 128:(i + 1) * 128, :])
                    am = p0n.tile([128, 1], FP32, tag="am_w")
                    nc.vector.tensor_reduce(am[:], wt[:, :wcols],
                                            axis=mybir.AxisListType.X,
                                            op=OP.max, apply_absolute_value=True)
                    if i == 0:
                        nc.vector.tensor_copy(acc[:], am[:])
                    else:
                        nc.vector.tensor_tensor(acc[:], acc[:], am[:], OP.max)
                nc.sync.dma_start(amax_stage[:, wi:wi + 1], acc[:])
            rowv = singles.tile([1, 256], FP32, tag="rowv")
            nc.sync.dma_start(
                rowv[:], amax_stage[:].rearrange("p c -> (p c)").unsqueeze(0))
            sc2 = singles.tile([1, 2], FP32, tag="sc2")
            nc.vector.tensor_reduce(
                sc2[:], rowv[:].rearrange("p (a b) -> p b a", b=2),
                axis=mybir.AxisListType.X, op=OP.max)
            nc.sync.dma_start(scalars_loc[:], sc2[:])
            nc.gpsimd.collective_compute(
                "AllReduce", OP.max, replica_groups=RG,
                ins=[scalars_loc[:].opt()], outs=[scales_sh[:].opt()])

            # ---- derive per-tensor scale scalars ----
            samax = singles.tile([128, 2], FP32, tag="samax")
            sc_ap = scales_sh[:]
            bcast = bass.AP(tensor=sc_ap.tensor, offset=sc_ap.offset,
                            ap=[[0, 128]] + list(sc_ap.ap)[1:])
            nc.gpsimd.dma_start(samax[:], bcast)
            ts_w = singles.tile([128, 2], FP32, tag="ts_w")
            nc.vector.tensor_scalar(ts_w[:], samax[:], inv2688, None, OP.mult)
            d_w = singles.tile([128, 2], FP32, tag="d_w")
            nc.vector.tensor_scalar(d_w[:], ts_w[:], 6.0, None, OP.mult)
            rdw = singles.tile([128, 2], FP32, tag="rdw")
            nc.vector.reciprocal(rdw[:], d_w[:])
            s_h = singles.tile([128, 1], FP32, tag="s_h")
            nc.vector.tensor_scalar(s_h[:], ts_w[:, 0:1], float(isc), None, OP.mult)
            s_o = singles.tile([128, 1], FP32, tag="s_o")
            nc.vector.tensor_scalar(s_o[:], ts_w[:, 1:2], float(hsc), None, OP.mult)
            s_h2 = singles.tile([128, 1], FP32, tag="s_h2")
            nc.vector.tensor_scalar(s_h2[:], s_h[:],
                                    float(f32(1.0) / f32(np.sqrt(np.float64(2.0)))),
                                    None, OP.mult)

            # ---- quantize w1 ----
            for i in range(HSH // 128):
                wt = p0src.tile([128, D_IN], FP32, tag="xt")
                nc.sync.dma_start(wt[:], w1_sh[i * 128:(i + 1) * 128, :])
                wo = p0src.tile([128, D_IN], BF16, tag="xo")
                for c in range(4):
                    sl = slice(c * 1024, (c + 1) * 1024)
                    _emit_quant(nc, mybir, p0f, p0b, p0n, biases,
                                wt[:, sl], wo[:, sl], rdw[:, 0:1], ts_w[:, 0:1], 1024)
                nc.sync.dma_start(w1q[i * 128:(i + 1) * 128, :], wo[:])

        # ================= Phase 1 =================
        with tc.tile_pool(name="w1T", bufs=1) as w1T_pool, \
             tc.tile_pool(name="xb", bufs=2) as xb_pool, \
             tc.tile_pool(name="q1f", bufs=2) as q1f, \
             tc.tile_pool(name="q1b", bufs=2) as q1b, \
             tc.tile_pool(name="q1n", bufs=2) as q1n, \
             tc.tile_pool(name="w2s", bufs=1) as w2s, \
             tc.tile_pool(name="w2o", bufs=2) as w2o, \
             tc.tile_pool(name="ps1", bufs=8, space="PSUM") as ps1:
            w1T = w1T_pool.tile([128, NK1, HSH], BF16, tag="w1T")
            for k in range(NK1):
                nc.sync.dma_start(w1T[:, k, :], w1q[:, k * 128:(k + 1) * 128],
                                  transpose=True)
            for t in range(NBT):
                g0 = t * 128
                ci, off = divmod(g0, BSH)
                xb = xb_pool.tile([128, NK1, 128], BF16, tag="xb")
                nc.sync.dma_start(
                    xb[:],
                    xqT_full[ci * D_IN:(ci + 1) * D_IN, off:off + 128]
                    .rearrange("(k p) c -> p k c", p=128))
                pss = [ps1.tile([128, 512], FP32, name="ps", tag="ps") for _ in range(4)]
                for k in range(NK1):
                    for n in range(4):
                        nc.tensor.matmul(
                            pss[n][:], lhsT=xb[:, k, :],
                            rhs=w1T[:, k, n * 512:(n + 1) * 512],
                            start=(k == 0), stop=(k == NK1 - 1))
                for n in range(4):
                    ho = q1b.tile([128, 512], BF16, tag="q_ho")
                    if USE_ERF:
                        # v = h*(1+erf(h/sqrt2)) = 2*gelu(h); the /2 is folded
                        # into the quantizer constants (exact pow-2 scalings).
                        et = q1f.tile([128, 512], FP32, tag="q_g")
                        nc.scalar.activation(et[:], pss[n][:], AF.Erf,
                                             scale=s_h2[:])
                        nc.vector.tensor_scalar(et[:], et[:], 1.0, None, OP.add)
                        hm = q1f.tile([128, 512], FP32, tag="q_hm")
                        nc.scalar.activation(hm[:], pss[n][:], AF.Copy,
                                             scale=s_h[:])
                        v = q1f.tile([128, 512], FP32, tag="q_v")
                        nc.vector.tensor_tensor(v[:], hm[:], et[:], OP.mult)
                        _emit_quant(nc, mybir, q1f, q1b, q1n, biases,
                                    v[:], ho[:], c1h2, float(2.0 * hsc), 512)
                    else:
                        g = q1f.tile([128, 512], FP32, tag="q_g")
                        nc.scalar.activation(g[:], pss[n][:], AF.Gelu,
                                             scale=s_h[:])
                        _emit_quant(nc, mybir, q1f, q1b, q1n, biases,
                                    g[:], ho[:], c1h, float(hsc), 512)
                    nc.sync.dma_start(
                        hq[g0:g0 + 128, n * 512:(n + 1) * 512], ho[:])
                # interleave one w2 row-tile quantization per two b-tiles
                if t % 2 == 1:
                    wi = t // 2
                    wt2 = w2s.tile([128, HSH], FP32, tag="wt2")
                    nc.sync.dma_start(wt2[:], w2_sh[wi * 128:(wi + 1) * 128, :])
                    wo2 = w2o.tile([128, HSH], BF16, tag="wo2")
                    for c in range(4):
                        sl = slice(c * 512, (c + 1) * 512)
                        _emit_quant(nc, mybir, q1f, q1b, q1n, biases,
                                    wt2[:, sl], wo2[:, sl],
                                    rdw[:, 1:2], ts_w[:, 1:2], 512)
                    nc.sync.dma_start(w2q[wi * 128:(wi + 1) * 128, :], wo2[:])

        # ================= Phase 2 =================
        with tc.tile_pool(name="w2T", bufs=1) as w2T_pool, \
             tc.tile_pool(name="hT", bufs=2) as hT_pool, \
             tc.tile_pool(name="osb", bufs=2) as osb, \
             tc.tile_pool(name="ps2", bufs=8, space="PSUM") as ps2:
            w2T = w2T_pool.tile([128, NK2, D_OUT], BF16, tag="w2T")
            for k in range(NK2):
                nc.sync.dma_start(w2T[:, k, :], w2q[:, k * 128:(k + 1) * 128],
                                  transpose=True)
            for sb in range(NSB):
                r0 = sb * SB
                hT = hT_pool.tile([128, NK2, SB], BF16, tag="hT")
                for k in range(NK2):
                    nc.sync.dma_start(hT[:, k, :],
                                      hq[r0:r0 + SB, k * 128:(k + 1) * 128],
                                      transpose=True)
                for b in range(SB // 128):
                    row = r0 + b * 128
                    c = row // RSROWS
                    crow = row % RSROWS
                    pss = [ps2.tile([128, 512], FP32, name="ps2", tag="ps2")
                           for _ in range(8)]
                    for k in range(NK2):
                        for n in range(8):
                            nc.tensor.matmul(
                                pss[n][:], lhsT=hT[:, k, b * 128:(b + 1) * 128],
                                rhs=w2T[:, k, n * 512:(n + 1) * 512],
                                start=(k == 0), stop=(k == NK2 - 1))
                    ot = osb.tile([128, D_OUT], FP32, tag="ot")
                    for n in range(8):
                        nc.scalar.activation(ot[:, n * 512:(n + 1) * 512],
                                             pss[n][:], AF.Copy, scale=s_o[:])
                    nc.sync.dma_start(parts[c][crow:crow + 128, :], ot[:])
                if sb % 4 == 3:
                    c = sb // 4
                    nc.gpsimd.collective_compute(
                        "ReduceScatter", OP.add, replica_groups=RG,
                        ins=[parts[c][:].opt()], outs=[rsouts[c][:].opt()])
                    nc.sync.dma_start(out_sh[c * 128:(c + 1) * 128, :],
                                      rsouts[c][:])
    nc.compile()
    return nc


def _get_built(isc, hsc):
    key = (float(isc), float(hsc), USE_ERF)
    if key not in _BUILT:
        _BUILT[key] = _build(float(isc), float(hsc))
    return _BUILT[key]


def run(x, w1, w2, input_scale, hidden_scale, trace=False):
    from concourse import bass_utils
    isc = float(np.asarray(input_scale).reshape(-1)[0])
    hsc = float(np.asarray(hidden_scale).reshape(-1)[0])
    nc = _get_built(isc, hsc)
    x = np.ascontiguousarray(x, dtype=np.float32)
    w1 = np.ascontiguousarray(w1, dtype=np.float32)
    w2 = np.ascontiguousarray(w2, dtype=np.float32)
    in_maps = []
    for c in range(NCORES):
        in_maps.append({
            "x_sh": x[c * BSH:(c + 1) * BSH, :],
            "w1_sh": np.ascontiguousarray(w1[c * HSH:(c + 1) * HSH, :]),
            "w2_sh": np.ascontiguousarray(w2[:, c * HSH:(c + 1) * HSH]),
        })
    res = bass_utils.run_bass_kernel_spmd(
        nc, in_maps, core_ids=list(range(NCORES)), trace=trace)
    out = np.empty((B, D_OUT), dtype=np.float32)
    for r in range(NCORES):
        o = res.results[r]["out_sh"]
        for c in range(RSCH):
            out[c * RSROWS + r * 128:c * RSROWS + (r + 1) * 128, :] = \
                o[c * 128:(c + 1) * 128, :]
    return out, res


def kernel(x, w1, w2, input_scale, hidden_scale):
    out, _ = run(x, w1, w2, input_scale, hidden_scale, trace=False)
    return out
